# revision 17
# baseline (speedup 1.0000x reference)
"""DenseMissing (GMM-imputed dense layer + expected ReLU) Trainium2 kernel.

Math (per row n, component c, output unit u):
  mask m[n,p] = isnan(x); xs = nan_to_0(x)
  loglik[n,c] = (xs^2)@a + xs@b - M@d + sum_d  (a=-1/(2v), b=mu/v, d=mu^2/(2v)+log(2 pi v)/2)
  p[n,c] = softmax(logits + loglik)
  mean_c  = xs@K + M@(mu_c*K)        (+ bias)
  var_c   = M@(var_c*K^2)
  out[n,u] = sum_c p_c * [ s*phi(w) + mean*Phi(w) ],  s=sqrt(var), w=mean/s
  with phi(w)=exp(-w^2/2)/sqrt(2pi); Phi via tanh-gelu approx
       Phi(w) ~= 0.5 + 0.5*tanh(ga*(w + gb*w^3))

Sharding: rows N split across 8 cores (data parallel); small params replicated.

Host<->device traffic over the PJRT tunnel dominates wall time (~40MB/s), so
I/O is minimized: x ships as fp16 (NaN survives the cast), the output returns
as fp16, the 7 per-component weight matrices are expanded on-device from the
[P,U] kernel, and the donated output buffer is recycled from the previous
call instead of shipping fresh zeros.
"""

import sys

sys.path.insert(0, "/opt/trn_rl_repo")

import numpy as np

import concourse.bass as bass
import concourse.mybir as mybir
import concourse.tile as tile
from concourse import bacc
from concourse.bass_utils import run_bass_kernel_spmd
from concourse.masks import make_identity
from concourse.dve_ops import RECIPROCAL_APPROX_FAST

F32 = mybir.dt.float32
F16 = mybir.dt.float16
F32R = mybir.dt.float32r
ALU = mybir.AluOpType
ACTF = mybir.ActivationFunctionType

N, P, C, U = 65536, 256, 3, 512
NCORES = 8
NLOC = N // NCORES
BLK = 128
PCH = P // 128  # p chunks (2)

PI = 3.14159265359  # matches reference
GA = 0.7978845608028654  # sqrt(2/pi)
GB = 0.044715
INV_SQRT_2PI = 0.3989422804014327
LN_INV_SQRT_2PI = -0.9189385332046727


U16 = mybir.dt.uint16

# Output wire format: f16 with PACK_DROP mantissa LSBs dropped, bit-packed.
# PACK_DROP=4 -> 12-bit values, 3 u16 words per 4 values (48MB wire);
# PACK_DROP=6 -> 10-bit values, 5 u16 words per 8 values (40MB wire);
# PACK_DROP=0 -> plain f16 (64MB wire).
PACK_DROP = 4


def packed_cols(drop):
    return U * (16 - drop) // 16


def unpack_out(pk, drop):
    """Host-side unpack of the packed u16 wire format -> f32 [rows, U]."""
    if not drop:
        return np.asarray(pk).astype(np.float32)
    rows = pk.shape[0]
    out = np.empty((rows, U), np.uint16)
    if drop == 4:
        p0, p1, p2 = pk[:, 0::3], pk[:, 1::3], pk[:, 2::3]
        t = [
            p0 & 0xFFF,
            (p0 >> 12) | ((p1 & 0xFF) << 4),
            (p1 >> 8) | ((p2 & 0xF) << 8),
            p2 >> 4,
        ]
        for i, ti in enumerate(t):
            out[:, i::4] = ti << 4
    elif drop == 6:
        p = [pk[:, i::5] for i in range(5)]
        t = [
            p[0] & 0x3FF,
            (p[0] >> 10) | ((p[1] & 0xF) << 6),
            (p[1] >> 4) & 0x3FF,
            (p[1] >> 14) | ((p[2] & 0xFF) << 2),
            (p[2] >> 8) | ((p[3] & 0x3) << 8),
            (p[3] >> 2) & 0x3FF,
            (p[3] >> 12) | ((p[4] & 0x3F) << 4),
            p[4] >> 6,
        ]
        for i, ti in enumerate(t):
            out[:, i::8] = ti << 6
    else:
        raise ValueError(drop)
    return out.view(np.float16).astype(np.float32)


def build_nc(n_loc=NLOC, super_=7, has_bias=False, mm_dt=F32R,
             fp16=True, use_absrsqrt=True, gp_folds=True, q_on_act=False,
             pipelined=True, prio_off=200, loop_reps=None,
             pack_drop=PACK_DROP):
    """Build the per-core bass program. Each core gets rows [n_loc, P]."""
    nb = n_loc // BLK
    nc = bacc.Bacc(
        "TRN2",
        target_bir_lowering=False,
        debug=False,
        num_devices=NCORES,
    )

    x_d = nc.dram_tensor("x", [n_loc, P], F16, kind="ExternalInput").ap()
    # knl: the dense-layer kernel K [P, U]; per-component weights are
    # expanded on-device (K, mu_c*K, var_c*K^2) to avoid shipping 7x.
    knl_d = nc.dram_tensor("knl", [P, U], F32, kind="ExternalInput").ap()
    # cmcv: [P, 8] = [cm0 cm1 cm2 cv0 cv1 cv2 pad pad]
    cmcv_d = nc.dram_tensor("cmcv", [P, 8], F32, kind="ExternalInput").ap()
    # llw: [P, 9] = [b | a | -d]
    llw_d = nc.dram_tensor("llw", [P, 9], F32, kind="ExternalInput").ap()
    # cvec: [1, 4] = logits + sum_d (3) + pad
    cvec_d = nc.dram_tensor("cvec", [1, 4], F32, kind="ExternalInput").ap()
    if has_bias:
        biasu_d = nc.dram_tensor("biasu", [1, U], F32, kind="ExternalInput").ap()
    if pack_drop:
        out_d = nc.dram_tensor(
            "out", [n_loc, packed_cols(pack_drop)], U16, kind="ExternalOutput"
        ).ap()
    else:
        out_d = nc.dram_tensor("out", [n_loc, U], F16, kind="ExternalOutput").ap()

    from contextlib import ExitStack

    with tile.TileContext(nc) as tc, ExitStack() as ctx:
        singles = ctx.enter_context(tc.tile_pool(name="singles", bufs=1))
        xp = ctx.enter_context(tc.tile_pool(name="xp", bufs=3))
        clean = ctx.enter_context(tc.tile_pool(name="clean", bufs=2))
        tp_ps = ctx.enter_context(tc.tile_pool(name="tp_ps", bufs=1, space="PSUM"))
        mv_ps = ctx.enter_context(tc.tile_pool(name="mv_ps", bufs=1, space="PSUM"))
        xfer_p = ctx.enter_context(tc.tile_pool(name="xfer_p", bufs=2))
        sph = ctx.enter_context(tc.tile_pool(name="sph", bufs=super_ + 1))
        sqp = ctx.enter_context(tc.tile_pool(name="sqp", bufs=2))
        work = ctx.enter_context(tc.tile_pool(name="work", bufs=1))
        wsm = ctx.enter_context(tc.tile_pool(name="wsm", bufs=8))
        outp = ctx.enter_context(tc.tile_pool(name="outp", bufs=3))

        # --- persistent tiles ---
        # On-device weight expansion: wt[k][:, 0:U]=K, [(1+c)U:(2+c)U]=cm_c*K,
        # [(4+c)U:(5+c)U]=cv_c*K^2, built once per launch from knl/cmcv.
        kk = []
        cc = []
        for k in range(PCH):
            t = singles.tile([128, U], F32, tag=f"kk{k}")
            nc.sync.dma_start(out=t, in_=knl_d[k * 128 : (k + 1) * 128, :])
            kk.append(t)
            t2 = singles.tile([128, 8], F32, tag=f"cc{k}")
            nc.sync.dma_start(out=t2, in_=cmcv_d[k * 128 : (k + 1) * 128, :])
            cc.append(t2)
        wt = []
        for k in range(PCH):
            # ACT ops write the f32r tile directly (rounded-to-f32r as the
            # BIR verifier requires); cmcv cols 3:6 hold sqrt(cv) so
            # cv_c*K^2 = Square(sqrt(cv_c)*K) is one scaled-square op.
            t = singles.tile([128, 7 * U], mm_dt, tag=f"wt{k}")
            nc.scalar.copy(t[:, 0:U], kk[k])
            for c in range(C):
                nc.scalar.activation(
                    t[:, (1 + c) * U : (2 + c) * U], kk[k],
                    ACTF.Copy, scale=cc[k][:, c : c + 1],
                )
            for c in range(C):
                nc.scalar.activation(
                    t[:, (4 + c) * U : (5 + c) * U], kk[k],
                    ACTF.Square, scale=cc[k][:, 3 + c : 4 + c],
                )
            wt.append(t)
        llw = []
        for k in range(PCH):
            t = singles.tile([128, 9], F32, tag=f"llw{k}")
            nc.sync.dma_start(out=t, in_=llw_d[k * 128 : (k + 1) * 128, :])
            llw.append(t)
        cvec = singles.tile([128, 4], F32, tag="cvec")
        cvec_b = bass.AP(
            tensor=cvec_d.tensor,
            offset=cvec_d.offset,
            ap=[[0, 128], cvec_d.ap[1]],
        )
        nc.sync.dma_start(out=cvec, in_=cvec_b)
        ident = singles.tile([128, 128], F32, tag="ident")
        make_identity(nc, ident)
        zeros = singles.tile([128, P], F32, tag="zeros")
        nc.vector.memset(zeros, 0.0)
        cb_exp = singles.tile([128, 1], F32, tag="cb_exp")
        nc.vector.memset(cb_exp, LN_INV_SQRT_2PI)
        cb_zero = singles.tile([128, 1], F32, tag="cb_zero")
        nc.vector.memset(cb_zero, 0.0)
        if has_bias:
            ones1 = singles.tile([1, 128], F32, tag="ones1")
            nc.vector.memset(ones1, 1.0)
            bias_sb = singles.tile([1, U], F32, tag="bias_sb")
            nc.sync.dma_start(out=bias_sb, in_=biasu_d)

        def phase_a(ib):
            """load, clean, transpose, matmuls, S-phase (sqrt-set ACT ops).

            Returns dict of SBUF tiles for phase E."""
            x16 = xp.tile([BLK, P], F16, tag="x16")
            nc.sync.dma_start(out=x16, in_=x_d[ib * BLK : (ib + 1) * BLK, :])
            x_sb = xp.tile([BLK, P], F32, tag="x")
            nc.scalar.copy(x_sb, x16)  # f16 -> f32 upcast, NaN preserved

            m_sb = clean.tile([BLK, P], F32, tag="m")
            xs_sb = clean.tile([BLK, P], F32, tag="xs")
            # m = (x != x) -> 1.0 at NaN
            nc.vector.tensor_tensor(m_sb, x_sb, x_sb, ALU.not_equal)
            # xs = where(m < 0.5, x, 0) in one validated custom DVE op
            from concourse.dve_ops import TENSOR_MASK

            nc.vector._custom_dve(
                TENSOR_MASK, out=xs_sb, in0=x_sb, in1=m_sb, s0=0.5, imm2=0.0
            )

            # transposes -> one PSUM bank [xsT0|xsT1|mT0|mT1]
            tp = tp_ps.tile([128, 512], F32, tag="tp")
            for k in range(PCH):
                nc.tensor.transpose(
                    tp[:, k * 128 : (k + 1) * 128],
                    xs_sb[:, k * 128 : (k + 1) * 128],
                    ident,
                )
            for k in range(PCH):
                nc.tensor.transpose(
                    tp[:, 256 + k * 128 : 256 + (k + 1) * 128],
                    m_sb[:, k * 128 : (k + 1) * 128],
                    ident,
                )
            xfer = xfer_p.tile([128, 512], mm_dt, tag="xfer")
            with tc.high_priority(offset=prio_off):
                nc.scalar.copy(xfer, tp)  # evacuate all 4 transposed chunks
            xsq = xfer_p.tile([128, 256], F32, tag="xsq")
            nc.scalar.square(xsq, xfer[:, 0:256])

            def xsT(k):
                return xfer[:, k * 128 : (k + 1) * 128]

            def mT(k):
                return xfer[:, 256 + k * 128 : 256 + (k + 1) * 128]

            MEAN = mv_ps.tile([128, C, U], F32, tag="MEAN")
            VAR = mv_ps.tile([128, C, U], F32, tag="VAR")
            LL = mv_ps.tile([128, 9], F32, tag="LL")

            # mean_c = xs@K + M@Kmu_c  (f32r), var_c = M@Kvar_c
            for k in range(PCH):
                for c in range(C):
                    nc.tensor.matmul(
                        MEAN[:, c, :],
                        xsT(k),
                        wt[k][:, 0:U],
                        start=(k == 0),
                        stop=False,
                    )
                nc.tensor.matmul(
                    LL[:, 0:3],
                    xfer[:, k * 128 : (k + 1) * 128].bitcast(F32),
                    llw[k][:, 0:3],
                    start=(k == 0),
                    stop=(k == PCH - 1),
                )
            for k in range(PCH):
                for c in range(C):
                    nc.tensor.matmul(
                        MEAN[:, c, :],
                        mT(k),
                        wt[k][:, (1 + c) * U : (2 + c) * U],
                        start=False,
                        stop=(k == PCH - 1) and not has_bias,
                    )
                for c in range(C):
                    nc.tensor.matmul(
                        VAR[:, c, :],
                        mT(k),
                        wt[k][:, (4 + c) * U : (5 + c) * U],
                        start=(k == 0),
                        stop=(k == PCH - 1),
                    )
                nc.tensor.matmul(
                    LL[:, 6:9],
                    xfer[:, 256 + k * 128 : 256 + (k + 1) * 128].bitcast(F32),
                    llw[k][:, 6:9],
                    start=(k == 0),
                    stop=(k == PCH - 1),
                )
            for k in range(PCH):
                nc.tensor.matmul(
                    LL[:, 3:6],
                    xsq[:, k * 128 : (k + 1) * 128],
                    llw[k][:, 3:6],
                    start=(k == 0),
                    stop=(k == PCH - 1),
                )
            if has_bias:
                for c in range(C):
                    nc.tensor.matmul(
                        MEAN[:, c, :],
                        ones1,
                        bias_sb,
                        start=False,
                        stop=True,
                    )

            # ---- evacuation phase ----
            EDT = mybir.dt.float16 if fp16 else F32
            MEANw = MEAN.rearrange("p c u -> p (c u)")
            VARw = VAR.rearrange("p c u -> p (c u)")
            mm = sph.tile([128, C * U], EDT, tag="mm")
            with tc.high_priority(offset=prio_off):
                nc.scalar.copy(mm, MEANw)
            if pipelined:
                # set-agnostic evacuation (Copy exists in every ACT table
                # set, so these never force a table load); the sqrt-set ACT
                # work happens later in a per-group batch.
                v32 = sph.tile([128, C * U], EDT, tag="v32")
                lle = sph.tile([128, 9], F32, tag="lle")
                with tc.high_priority(offset=prio_off):
                    nc.scalar.copy(v32, VARw)
                    nc.vector.tensor_copy(lle, LL)
                lg = sph.tile([128, 3], F32, tag="lg")
                l1 = sph.tile([128, 3], F32, tag="l1")
                nc.vector.tensor_tensor(l1, lle[:, 0:3], lle[:, 3:6], ALU.add)
                nc.vector.tensor_tensor(l1, l1, lle[:, 6:9], ALU.add)
                nc.vector.tensor_tensor(lg, l1, cvec[:, 0:3], ALU.add)
                return dict(v32=v32, mm=mm, lg=lg)
            if use_absrsqrt:
                r16 = sph.tile([128, C * U], EDT, tag="r16")
                nc.scalar.activation(r16, VARw, ACTF.Abs_reciprocal_sqrt,
                                     bias=cb_zero)
                sh = sph.tile([128, C * U], EDT, tag="sh")
                nc.vector.tensor_tensor(sh, VARw, r16, ALU.mult)
            else:
                s32 = sqp.tile([128, C * U], F32, tag="s32")
                nc.scalar.sqrt(s32, VARw)
                r16 = sph.tile([128, C * U], EDT, tag="r16")
                if fp16:
                    from concourse.dve_ops import RECIP_APPROX_FAST_CONSTS as _RC

                    nc.vector._custom_dve(
                        RECIPROCAL_APPROX_FAST, out=r16, in0=s32,
                        s0=_RC["s0"], s1=_RC["s1"], imm2=_RC["imm2"],
                    )
                else:
                    nc.vector.reciprocal_approx_fast(out=r16, in_=s32)
                sh = sph.tile([128, C * U], EDT, tag="sh")
                nc.vector.tensor_copy(sh, s32)
            lle = sph.tile([128, 9], F32, tag="lle")
            nc.scalar.copy(lle, LL)
            lg = sph.tile([128, 3], F32, tag="lg")
            l1 = sph.tile([128, 3], F32, tag="l1")
            nc.vector.tensor_tensor(l1, lle[:, 0:3], lle[:, 3:6], ALU.add)
            nc.vector.tensor_tensor(l1, l1, lle[:, 6:9], ALU.add)
            nc.vector.tensor_tensor(lg, l1, cvec[:, 0:3], ALU.add)
            return dict(sh=sh, mm=mm, r16=r16, lg=lg)

        def phase_s(d):
            """sqrt-set (or absrsqrt-set) batch producing r = rsqrt(v), s."""
            EDT = mybir.dt.float16 if fp16 else F32
            v32 = d["v32"]
            r16 = sph.tile([128, C * U], EDT, tag="r16")
            sh = sph.tile([128, C * U], EDT, tag="sh")
            if use_absrsqrt:
                nc.scalar.activation(r16, v32, ACTF.Abs_reciprocal_sqrt,
                                     bias=cb_zero)
                yield
                nc.vector.tensor_tensor(sh, v32, r16, ALU.mult)
            else:
                s32 = sqp.tile([128, C * U], F32, tag="s32")
                nc.scalar.sqrt(s32, v32)
                if fp16:
                    from concourse.dve_ops import RECIP_APPROX_FAST_CONSTS as _RC

                    nc.vector._custom_dve(
                        RECIPROCAL_APPROX_FAST, out=r16, in0=s32,
                        s0=_RC["s0"], s1=_RC["s1"], imm2=_RC["imm2"],
                    )
                else:
                    nc.vector.reciprocal_approx_fast(out=r16, in_=s32)
                yield
                nc.vector.tensor_copy(sh, s32)
            d["r16"] = r16
            d["sh"] = sh

        def phase_e(ib, d):
            """exp-set ACT ops + DVE chain + output DMA."""
            EDT = mybir.dt.float16 if fp16 else F32
            sh16, mm, r16, lg = d["sh"], d["mm"], d["r16"], d["lg"]
            # softmax over C=3
            mx = wsm.tile([128, 1], F32, tag="wsm")
            nc.vector.tensor_reduce(mx, lg, mybir.AxisListType.X, ALU.max)
            shl = wsm.tile([128, 3], F32, tag="wsm")
            nc.vector.tensor_scalar(shl, lg, mx, None, ALU.subtract)
            ex = wsm.tile([128, 3], F32, tag="wsm")
            nc.scalar.activation(ex, shl, ACTF.Exp, bias=cb_zero)
            sm = wsm.tile([128, 1], F32, tag="wsm")
            nc.vector.tensor_reduce(sm, ex, mybir.AxisListType.X, ALU.add)
            ism = wsm.tile([128, 1], F32, tag="wsm")
            nc.vector.reciprocal(ism, sm)
            p = wsm.tile([128, 3], F32, tag="wsm")
            nc.vector.tensor_scalar(p, ex, ism, None, ALU.mult)
            ph = wsm.tile([128, 3], F32, tag="wsm")
            nc.vector.tensor_scalar(ph, p, 0.5, None, ALU.mult)
            yield

            w = work.tile([128, C * U], EDT, tag="w")
            nc.vector.tensor_tensor(w, mm, r16, ALU.mult)
            yield
            q = work.tile([128, C * U], EDT, tag="q")
            if q_on_act:
                nc.scalar.square(q, w)
            else:
                nc.vector.tensor_tensor(q, w, w, ALU.mult)
            yield
            e = work.tile([128, C * U], EDT, tag="e")
            nc.scalar.activation(e, q, ACTF.Exp, bias=cb_exp, scale=-0.5)
            u1 = work.tile([128, C * U], EDT, tag="u1")
            nc.vector.tensor_scalar(u1, q, GA * GB, GA, ALU.mult, ALU.add)
            yield
            z = work.tile([128, C * U], EDT, tag="z")
            nc.vector.tensor_tensor(z, u1, w, ALU.mult)
            yield
            T = work.tile([128, C * U], EDT, tag="T")
            nc.scalar.activation(T, z, ACTF.Tanh, bias=cb_zero)
            yield

            ep = work.tile([128, C, U], EDT, tag="ep")
            Pp = work.tile([128, C, U], EDT, tag="Pp")
            for c in range(C):
                nc.vector.tensor_scalar(
                    ep[:, c, :],
                    e[:, c * U : (c + 1) * U],
                    p[:, c : c + 1],
                    None,
                    ALU.mult,
                )
                nc.vector.tensor_scalar(
                    Pp[:, c, :],
                    T[:, c * U : (c + 1) * U],
                    ph[:, c : c + 1],
                    ph[:, c : c + 1],
                    ALU.mult,
                    ALU.add,
                )
            epw = ep.rearrange("p c u -> p (c u)")
            Ppw = Pp.rearrange("p c u -> p (c u)")
            yield
            t1 = work.tile([128, C * U], EDT, tag="t1")
            nc.vector.tensor_tensor(t1, sh16, epw, ALU.mult)
            t2 = work.tile([128, C * U], EDT, tag="t2")
            nc.vector.tensor_tensor(t2, mm, Ppw, ALU.mult)
            yield
            eng = nc.gpsimd if gp_folds else nc.vector
            t12 = work.tile([128, C * U], EDT, tag="t12")
            eng.tensor_tensor(t12, t1, t2, ALU.add)
            yield
            o1 = work.tile([BLK, U], EDT, tag="o1")
            eng.tensor_tensor(o1, t12[:, 0:U], t12[:, U : 2 * U], ALU.add)
            yield
            ob = outp.tile([BLK, U], F16, tag="ob")
            eng.tensor_tensor(ob, o1, t12[:, 2 * U : 3 * U], ALU.add)
            if not pack_drop:
                nc.sync.dma_start(
                    out=out_d[ib * BLK : (ib + 1) * BLK, :], in_=ob
                )
                return
            # Bit-pack the f16 output to (16-pack_drop)-bit values. All ALU
            # work stays in u16 lanes (exact in the DVE's f32 datapath);
            # shift-lefts that could exceed 2^16 are masked first (fused
            # and+shift in one tensor_scalar).
            obu = ob.bitcast(U16)
            tr = outp.tile([BLK, U], U16, tag="tr")
            half = 1 << (pack_drop - 1)
            # round-to-nearest: +half then >>drop (separate instructions —
            # ALU add runs in fp32 lanes, shifts are integer ops)
            nc.vector.tensor_scalar(tr, obu, half, None, ALU.add)
            tq = outp.tile([BLK, U], U16, tag="tq")
            nc.vector.tensor_scalar(
                tq, tr, pack_drop, None, ALU.logical_shift_right
            )
            pk = outp.tile([BLK, packed_cols(pack_drop)], U16, tag="pk")

            def shl(dst, src, mask, sh):
                if mask is not None:
                    nc.vector.tensor_scalar(
                        dst, src, mask, sh, ALU.bitwise_and,
                        ALU.logical_shift_left,
                    )
                else:
                    nc.vector.tensor_scalar(
                        dst, src, sh, None, ALU.logical_shift_left
                    )

            def shr(dst, src, sh):
                nc.vector.tensor_scalar(
                    dst, src, sh, None, ALU.logical_shift_right
                )

            if pack_drop == 4:
                t = [tq[:, i::4] for i in range(4)]
                p = [pk[:, j::3] for j in range(3)]
                w = U // 4
                a = outp.tile([BLK, w], U16, tag="pka")
                b = outp.tile([BLK, w], U16, tag="pkb")
                shl(a, t[1], 0xF, 12)
                nc.vector.tensor_tensor(p[0], t[0], a, ALU.bitwise_or)
                shr(a, t[1], 4)
                shl(b, t[2], 0xFF, 8)
                nc.vector.tensor_tensor(p[1], a, b, ALU.bitwise_or)
                shr(a, t[2], 8)
                shl(b, t[3], None, 4)
                nc.vector.tensor_tensor(p[2], a, b, ALU.bitwise_or)
            elif pack_drop == 6:
                t = [tq[:, i::8] for i in range(8)]
                p = [pk[:, j::5] for j in range(5)]
                w = U // 8
                a = outp.tile([BLK, w], U16, tag="pka")
                b = outp.tile([BLK, w], U16, tag="pkb")
                shl(a, t[1], 0x3F, 10)
                nc.vector.tensor_tensor(p[0], t[0], a, ALU.bitwise_or)
                shr(a, t[1], 6)
                shl(b, t[2], None, 4)
                nc.vector.tensor_tensor(a, a, b, ALU.bitwise_or)
                shl(b, t[3], 0x3, 14)
                nc.vector.tensor_tensor(p[1], a, b, ALU.bitwise_or)
                shr(a, t[3], 2)
                shl(b, t[4], 0xFF, 8)
                nc.vector.tensor_tensor(p[2], a, b, ALU.bitwise_or)
                shr(a, t[4], 8)
                shl(b, t[5], None, 2)
                nc.vector.tensor_tensor(a, a, b, ALU.bitwise_or)
                shl(b, t[6], 0xF, 12)
                nc.vector.tensor_tensor(p[3], a, b, ALU.bitwise_or)
                shr(a, t[6], 4)
                shl(b, t[7], None, 6)
                nc.vector.tensor_tensor(p[4], a, b, ALU.bitwise_or)
            else:
                raise ValueError(pack_drop)
            nc.sync.dma_start(out=out_d[ib * BLK : (ib + 1) * BLK, :], in_=pk)

        import contextlib

        loop_cm = (
            tc.For_i(0, loop_reps, 1) if loop_reps else contextlib.nullcontext()
        )

        def run_rr(gens):
            gens = list(gens)
            while gens:
                nxt = []
                for gi in gens:
                    try:
                        next(gi)
                        nxt.append(gi)
                    except StopIteration:
                        pass
                gens = nxt

        ctx.enter_context(loop_cm)
        if not pipelined:
            for g0 in range(0, nb, super_):
                g = range(g0, min(g0 + super_, nb))
                ds = [phase_a(ib) for ib in g]
                run_rr([phase_e(ib, d) for ib, d in zip(g, ds)])
        else:
            groups = [
                list(range(g0, min(g0 + super_, nb)))
                for g0 in range(0, nb, super_)
            ]
            ds = {}
            prev = None
            for g in groups:
                if prev is None:
                    for ib in g:
                        ds[ib] = phase_a(ib)
                    prev = g
                    continue
                run_rr([phase_s(ds[ib]) for ib in prev])

                def _e_then_a(i, ib):
                    yield from phase_e(ib, ds.pop(ib))
                    if i < len(g):
                        ds[g[i]] = phase_a(g[i])

                run_rr([_e_then_a(i, ib) for i, ib in enumerate(prev)])
                for i in range(len(prev), len(g)):
                    ds[g[i]] = phase_a(g[i])
                prev = g
            run_rr([phase_s(ds[ib]) for ib in prev])
            run_rr([phase_e(ib, ds.pop(ib)) for ib in prev])

    nc.compile()
    return nc


def host_small(component_means, component_vars, component_logits):
    """Tiny host-side precompute: loglik weights + logit offsets."""
    cm = np.asarray(component_means, np.float64)
    cv = np.asarray(component_vars, np.float64)
    a = -0.5 / cv
    b = cm / cv
    d = -0.5 * cm**2 / cv - 0.5 * np.log(2.0 * PI * cv)
    llw = np.concatenate([b, a, -d], axis=1).astype(np.float32)
    cvec = np.zeros((1, 4), np.float32)
    cvec[0, :3] = (np.asarray(component_logits, np.float64) + d.sum(0)).astype(
        np.float32
    )
    cmcv = np.zeros((P, 8), np.float32)
    cmcv[:, 0:3] = cm.astype(np.float32)
    cmcv[:, 3:6] = np.sqrt(cv).astype(np.float32)
    return llw, cvec, cmcv


_NC_CACHE = {}
_EXEC_CACHE = {}
_DONATE_STASH = {}
_PARAM_DEV_CACHE = {}


def _get_exec(nc, cache_key):
    """Build (or fetch) a jitted shard_map executor for nc, mirroring
    bass2jax.run_bass_via_pjrt's multi-core path but accepting global
    arrays directly so donation buffers can live on device."""
    if cache_key in _EXEC_CACHE:
        return _EXEC_CACHE[cache_key]
    import jax
    from jax.experimental.shard_map import shard_map
    from jax.sharding import Mesh, NamedSharding, PartitionSpec

    from concourse import bass2jax, mybir as _mybir
    bass2jax.install_neuronx_cc_hook()

    in_names, out_names, out_avals = [], [], []
    partition_name = (
        nc.partition_id_tensor.name if nc.partition_id_tensor else None
    )
    for alloc in nc.m.functions[0].allocations:
        if not isinstance(alloc, _mybir.MemoryLocationSet):
            continue
        name = alloc.memorylocations[0].name
        if alloc.kind == "ExternalInput":
            if name != partition_name:
                in_names.append(name)
        elif alloc.kind == "ExternalOutput":
            out_names.append(name)
            out_avals.append(
                jax.core.ShapedArray(
                    tuple(alloc.tensor_shape), _mybir.dt.np(alloc.dtype)
                )
            )
    n_params = len(in_names)
    n_outs = len(out_names)
    all_names = in_names + out_names
    if partition_name is not None:
        all_names = all_names + [partition_name]
    donate = tuple(range(n_params, n_params + n_outs))

    def _body(*args):
        operands = list(args)
        if partition_name is not None:
            operands.append(bass2jax.partition_id_tensor())
        outs = bass2jax._bass_exec_p.bind(
            *operands,
            out_avals=tuple(out_avals),
            in_names=tuple(all_names),
            out_names=tuple(out_names),
            lowering_input_output_aliases=(),
            sim_require_finite=True,
            sim_require_nnan=True,
            nc=nc,
        )
        return tuple(outs)

    devices = jax.devices()[:NCORES]
    assert len(devices) == NCORES
    mesh = Mesh(np.asarray(devices), ("core",))
    in_specs = (PartitionSpec("core"),) * (n_params + n_outs)
    out_specs = (PartitionSpec("core"),) * n_outs
    sharded = jax.jit(
        shard_map(
            _body, mesh=mesh, in_specs=in_specs, out_specs=out_specs,
            check_rep=False,
        ),
        donate_argnums=donate,
        keep_unused=True,
    )
    sh8 = NamedSharding(mesh, PartitionSpec("core"))
    entry = (sharded, in_names, out_names, out_avals, sh8)
    _EXEC_CACHE[cache_key] = entry
    return entry


def kernel(x, component_means, component_vars, component_logits, kernel, bias):
    import jax

    x = np.asarray(x, np.float32)
    bias = np.asarray(bias, np.float32)
    has_bias = bool(np.any(bias != 0))
    n_loc = x.shape[0] // NCORES
    key = (x.shape[0], has_bias)
    if key not in _NC_CACHE:
        _NC_CACHE[key] = build_nc(n_loc=n_loc, has_bias=has_bias)
    nc = _NC_CACHE[key]

    llw, cvec, cmcv = host_small(
        component_means, component_vars, component_logits
    )
    knl = np.ascontiguousarray(np.asarray(kernel, np.float32))

    try:
        sharded, in_names, out_names, out_avals, sh8 = _get_exec(nc, key)
        # Cast x to f16 per-shard, overlapping each slice's cast with the
        # previous slice's (async) h2d transfer.
        devs = jax.devices()[:NCORES]
        xparts = [
            jax.device_put(
                np.ascontiguousarray(
                    x[i * n_loc : (i + 1) * n_loc].astype(np.float16)
                ),
                devs[i],
            )
            for i in range(NCORES)
        ]
        x16 = jax.make_array_from_single_device_arrays(
            (x.shape[0], P), sh8, xparts
        )
        # Static params: keep device-resident copies; re-upload only if the
        # bytes changed (exact comparison — cheap, all under 1MB).
        srcs = {"knl": knl, "cmcv": cmcv, "llw": llw, "cvec": cvec}
        if has_bias:
            srcs["biasu"] = bias.reshape(1, U)
        blobs = {k: v.tobytes() for k, v in srcs.items()}
        cached = _PARAM_DEV_CACHE.get(key)
        if cached is None or cached[0] != blobs:
            rep = {
                k: jax.device_put(
                    np.concatenate([v] * NCORES, axis=0), sh8
                )
                for k, v in srcs.items()
            }
            _PARAM_DEV_CACHE[key] = (blobs, rep)
        else:
            rep = cached[1]
        args = []
        for name in in_names:
            if name == "x":
                args.append(x16)
            else:
                args.append(rep[name])
        # Donation buffers: recycle last call's output device buffers
        # (contents irrelevant — the kernel writes every element).
        for i, oname in enumerate(out_names):
            stash = _DONATE_STASH.pop((key, oname), None)
            if stash is None:
                aval = out_avals[i]
                z = np.zeros(
                    (NCORES * aval.shape[0],) + tuple(aval.shape[1:]),
                    aval.dtype,
                )
                stash = jax.device_put(z, sh8)
            args.append(stash)
        out_arrs = sharded(*args)
        # Fetch shard-by-shard; a worker thread unpacks shard i while the
        # (GIL-releasing) d2h copy of shard i+1 is in flight.
        from concurrent.futures import ThreadPoolExecutor

        res = np.empty((x.shape[0], U), np.float32)

        def _unpack_into(wire, start):
            res[start : start + wire.shape[0]] = unpack_out(wire, PACK_DROP)

        shards = sorted(
            out_arrs[0].addressable_shards,
            key=lambda s: s.index[0].start or 0,
        )
        with ThreadPoolExecutor(max_workers=1) as ex:
            futs = []
            for s in shards:
                wire = np.asarray(s.data)
                futs.append(
                    ex.submit(_unpack_into, wire, s.index[0].start or 0)
                )
            for f in futs:
                f.result()
        for oname, arr in zip(out_names, out_arrs):
            _DONATE_STASH[(key, oname)] = arr
        return res
    except Exception:
        _EXEC_CACHE.pop(key, None)
        _DONATE_STASH.clear()
        # Fallback: stock spmd runner (slower: host zeros + per-core concat).
        xh16 = np.ascontiguousarray(x.astype(np.float16))
        in_maps = []
        for i in range(NCORES):
            m = {
                "x": xh16[i * n_loc : (i + 1) * n_loc],
                "knl": knl,
                "cmcv": cmcv,
                "llw": llw,
                "cvec": cvec,
            }
            if has_bias:
                m["biasu"] = bias.reshape(1, U)
            in_maps.append(m)
        res = run_bass_kernel_spmd(nc, in_maps, core_ids=list(range(NCORES)))
        return unpack_out(
            np.concatenate([r["out"] for r in res.results], axis=0), PACK_DROP
        )


if __name__ == "__main__":
    # quick small-N CoreSim check (single core)
    from concourse.bass_interp import CoreSim

    rng = np.random.default_rng(0)
    n_test = 256
    xt = rng.standard_normal((n_test, P), dtype=np.float32)
    mask = rng.random((n_test, P)) < 0.15
    xt[mask] = np.nan
    cm = (0.5 * rng.standard_normal((P, C))).astype(np.float32)
    cv = rng.uniform(0.5, 1.5, (P, C)).astype(np.float32)
    cl = np.ones(C, np.float32)
    K = (rng.standard_normal((P, U)) / np.sqrt(P)).astype(np.float32)
    bias = np.zeros(U, np.float32)

    nc = build_nc(n_loc=n_test, super_=2, has_bias=False, use_absrsqrt=False)
    llw, cvec, cmcv = host_small(cm, cv, cl)
    sim = CoreSim(nc, require_finite=False, require_nnan=False)
    sim.tensor("x")[:] = xt.astype(np.float16)
    sim.tensor("knl")[:] = K
    sim.tensor("cmcv")[:] = cmcv
    sim.tensor("llw")[:] = llw
    sim.tensor("cvec")[:] = cvec
    sim.simulate()
    got = unpack_out(np.array(sim.tensor("out")), PACK_DROP).astype(np.float64)

    # numpy reference (on the fp16-rounded x the device sees)
    xr = xt.astype(np.float16).astype(np.float64)
    xs = np.where(mask, 0, xr)
    M = mask.astype(np.float64)
    a = -0.5 / cv.astype(np.float64)
    b = (cm / cv).astype(np.float64)
    d = (-0.5 * cm**2 / cv - 0.5 * np.log(2 * PI * cv)).astype(np.float64)
    ll = xs**2 @ a + xs @ b + d.sum(0)[None, :] - M @ d + cl[None, :]
    pw = np.exp(ll - ll.max(1, keepdims=True))
    pw /= pw.sum(1, keepdims=True)
    A = xs @ K.astype(np.float64)
    out = np.zeros((n_test, U))
    for c in range(C):
        mc = A + M @ (cm[:, c : c + 1] * K).astype(np.float64)
        vc = M @ (cv[:, c : c + 1] * K.astype(np.float64) ** 2)
        s = np.sqrt(vc)
        w = mc / s
        from scipy.special import erf as _erf

        vals = s * (
            np.exp(-0.5 * w * w) / np.sqrt(2 * PI)
            + 0.5 * w * (1 + _erf(w / np.sqrt(2)))
        )
        out += pw[:, c : c + 1] * vals
    rel = np.linalg.norm(got - out) / np.linalg.norm(out)
    print("rel err vs numpy ref:", rel)
    print("max abs diff:", np.abs(got - out).max())
